# revision 6
# baseline (speedup 1.0000x reference)
"""Multi-head attention (RoPE, causal) Trainium2 Bass kernel, 8-core SPMD.

Problem: B=2, S=2048, D=1024, H=16, DK=64, fp32, causal mask.

Sharding: core c handles batch b = c//4 and head group hg = c%4 (4 heads).
Each core computes Q/K/V projections for its 4 heads (column-sliced weights),
RoPE, causal attention, and a partial output projection (row-sliced Wo).
Host sums the 4 partial outputs per batch and adds the output bias.

Layout strategy (no on-device transposes):
  x^T [D, S] is precomputed on host; Q^T/K^T computed as [dk, S] tiles
  (weights stationary, x^T moving); scores computed transposed [k, q]
  (K^T stationary, Q^T moving); PV uses V in natural layout [k, dk+1]
  (stationary) with exp(scores^T) moving, accumulating attn^T [dk(+1), q];
  the ones column of V accumulates the softmax denominator. Normalization
  multiplies attn^T rows by broadcast 1/denom. Output projection uses
  attn^T as stationary and Wo as moving, producing natural [s, D] partials.

All matmul operands are float32r (TF32-like fast mode: 1 cycle/row at
moving-dim >= 256 vs 4 cycles/row for fp32): ~1e-4 relative L2 per matmul.
"""
import sys
sys.path.insert(0, "/opt/trn_rl_repo")
import math
import numpy as np

B, S, D, H, DK = 2, 2048, 1024, 16, 64
NCORES = 8
HPC = H // (NCORES // B)     # 4 heads per core
DHC = HPC * DK               # 256 attn dims per core
NPAIR = HPC // 2             # 2 head pairs per core
KC = D // 128                # 8 contraction chunks
NSB = S // 128               # 16 s-blocks / k-blocks
NQC = S // 512               # 4 q-chunks of 512

_BUILD_CACHE = {}


def _build(causal: bool):
    import concourse.tile as tile
    from concourse import bacc, mybir

    f32, f32r = mybir.dt.float32, mybir.dt.float32r
    MULT, ADD = mybir.AluOpType.mult, mybir.AluOpType.add
    EXP = mybir.ActivationFunctionType.Exp

    nc = bacc.Bacc(target_bir_lowering=False, trn_type="TRN2", debug=False)

    xT_d = nc.dram_tensor("xT", [D, S], f32r, kind="ExternalInput")
    wq_d = nc.dram_tensor("wq", [D, DHC], f32r, kind="ExternalInput")
    wk_d = nc.dram_tensor("wk", [D, DHC], f32r, kind="ExternalInput")
    wv_d = nc.dram_tensor("wv", [D, DHC], f32r, kind="ExternalInput")
    wo_d = nc.dram_tensor("wo", [DHC, D], f32r, kind="ExternalInput")
    bqk_d = nc.dram_tensor("bqk", [2, DHC], f32r, kind="ExternalInput")
    bv_d = nc.dram_tensor("bv", [1, DHC], f32r, kind="ExternalInput")
    ones_d = nc.dram_tensor("ones", [1, 512], f32r, kind="ExternalInput")
    psig_d = nc.dram_tensor("psig", [128, 128], f32r, kind="ExternalInput")
    rope_d = nc.dram_tensor("rope", [4, 128, S], f32, kind="ExternalInput")
    mdiag_d = nc.dram_tensor("mdiag", [128, 128], f32, kind="ExternalInput")
    out_d = nc.dram_tensor("out", [S, D], f32, kind="ExternalOutput")
    if not causal:
        maskT_d = nc.dram_tensor("maskT", [S, S], f32, kind="ExternalInput")

    with tile.TileContext(nc) as tc:
        with tc.tile_pool(name="const", bufs=1) as const_p, \
             tc.tile_pool(name="persist", bufs=1) as pers_p, \
             tc.tile_pool(name="ph2sb", bufs=1) as ph2_sb, \
             tc.tile_pool(name="ph3sb", bufs=1) as ph3_sb:

            # ---------- constants ----------
            ones_t = const_p.tile([1, 512], f32r, tag="ones")
            nc.sync.dma_start(out=ones_t, in_=ones_d[:])
            psig_t = const_p.tile([128, 128], f32r, tag="psig")
            nc.sync.dma_start(out=psig_t, in_=psig_d[:])
            bq_t = const_p.tile([1, DHC], f32r, tag="bq")
            nc.sync.dma_start(out=bq_t, in_=bqk_d[0:1, :])
            bk_t = const_p.tile([1, DHC], f32r, tag="bk")
            nc.sync.dma_start(out=bk_t, in_=bqk_d[1:2, :])
            bv_t = const_p.tile([1, DHC], f32r, tag="bv")
            nc.sync.dma_start(out=bv_t, in_=bv_d[:])
            mdiag_t = const_p.tile([128, 128], f32, tag="mdiag")
            nc.sync.dma_start(out=mdiag_t, in_=mdiag_d[:])

            # ---------- persistent activations ----------
            qt_pair = [pers_p.tile([128, S], f32r, tag=f"qt{p}", name=f"qt{p}") for p in range(NPAIR)]
            kt_pair = [pers_p.tile([128, S], f32r, tag=f"kt{p}", name=f"kt{p}") for p in range(NPAIR)]
            v_sb = [pers_p.tile([128, HPC, DK + 1], f32r, tag=f"v{i}", name=f"v{i}") for i in range(NSB)]
            attnT_sb = [pers_p.tile([128, S], f32r, tag=f"at{p}", name=f"at{p}") for p in range(NPAIR)]

            # =========================================================
            # Phase 1: projections + RoPE + V assembly
            # =========================================================
            with tc.tile_pool(name="ph1sb", bufs=1) as ph1_sb, \
                 tc.tile_pool(name="ph1ps", bufs=1, space="PSUM") as ph1_ps:

                rope_t = [ph1_sb.tile([128, S], f32, tag=f"rope{i}", name=f"rope{i}") for i in range(4)]
                for i in range(4):
                    nc.sync.dma_start(out=rope_t[i], in_=rope_d[i])
                w_t = {}
                for t_i, w_dram in ((0, wq_d), (1, wk_d)):
                    w_t[t_i] = [ph1_sb.tile([128, DHC], f32r, tag=f"w{t_i}_{kc}", name=f"w{t_i}_{kc}")
                                for kc in range(KC)]
                    for kc in range(KC):
                        nc.sync.dma_start(out=w_t[t_i][kc], in_=w_dram[128 * kc:128 * (kc + 1), :])
                wv_t = [ph1_sb.tile([128, DHC], f32r, tag=f"wv{kc}", name=f"wv{kc}")
                        for kc in range(KC)]
                for kc in range(KC):
                    nc.sync.dma_start(out=wv_t[kc], in_=wv_d[128 * kc:128 * (kc + 1), :])

                # loop q-chunks; stream xT slices once per chunk, reused by
                # Q/K projections (moving operand) and V projection (stationary)
                for qc in range(NQC):
                    ql, qh = 512 * qc, 512 * (qc + 1)
                    xq = [ph1_sb.tile([128, 512], f32r, tag="xq", bufs=10, name=f"xq{kc}_{qc}")
                          for kc in range(KC)]
                    for kc in range(KC):
                        nc.sync.dma_start(out=xq[kc], in_=xT_d[128 * kc:128 * (kc + 1), ql:qh])
                    pps = {}
                    for t_i in (0, 1):
                        for p in range(NPAIR):
                            pps[t_i, p] = ph1_ps.tile([128, 512], f32, tag="qtp",
                                                      bufs=4, name=f"pp{t_i}_{p}_{qc}")
                    for kc in range(KC):
                        for t_i in (0, 1):
                            for p in range(NPAIR):
                                nc.tensor.matmul(pps[t_i, p],
                                                 w_t[t_i][kc][:, 128 * p:128 * (p + 1)],
                                                 xq[kc], start=(kc == 0), stop=False)
                    for t_i in (0, 1):
                        cos_t, sin_t = rope_t[2 * t_i], rope_t[2 * t_i + 1]
                        dst_pair = qt_pair if t_i == 0 else kt_pair
                        bias_t = bq_t if t_i == 0 else bk_t
                        for p in range(NPAIR):
                            pp = pps[t_i, p]
                            nc.tensor.matmul(pp, bias_t[:, 128 * p:128 * (p + 1)],
                                             ones_t, start=False, stop=True)
                            # RoPE: dst = pp*cos + Psig @ (pp*sin_sig)
                            u_t = ph1_sb.tile([128, 512], f32r, tag="u", bufs=2)
                            nc.vector.tensor_tensor(out=u_t, in0=pp, in1=sin_t[:, ql:qh], op=MULT)
                            us = ph1_ps.tile([128, 512], f32, tag="usp", bufs=2)
                            nc.tensor.matmul(us, psig_t, u_t, start=True, stop=True)
                            dst = dst_pair[p][:, ql:qh]
                            nc.vector.tensor_tensor(out=dst, in0=pp, in1=cos_t[:, ql:qh], op=MULT)
                            nc.vector.tensor_tensor(out=dst, in0=us, in1=dst.bitcast(f32), op=ADD)
                    # V projection for the 4 s-blocks covered by this q-chunk
                    for r in range(4):
                        si = 4 * qc + r
                        vp = ph1_ps.tile([128, DHC + HPC], f32, tag="vp", bufs=2)
                        for kc in range(KC):
                            nc.tensor.matmul(vp[:, 0:DHC], xq[kc][:, 128 * r:128 * (r + 1)],
                                             wv_t[kc], start=(kc == 0), stop=False)
                        nc.tensor.matmul(vp[:, 0:DHC], ones_t[:, 0:128], bv_t,
                                         start=False, stop=True)
                        nc.tensor.matmul(vp[:, DHC:DHC + HPC], ones_t[:, 0:128],
                                         ones_t[:, 0:HPC], start=True, stop=True)
                        nc.scalar.copy(out=v_sb[si][:, :, 0:DK],
                                       in_=vp[:, 0:DHC].rearrange("p (h d) -> p h d", h=HPC))
                        nc.scalar.copy(out=v_sb[si][:, :, DK:DK + 1],
                                       in_=vp[:, DHC:DHC + HPC].rearrange("p (h o) -> p h o", h=HPC))

            # =========================================================
            # Phase 2: attention per head
            # =========================================================
            with tc.tile_pool(name="ph2ps", bufs=1, space="PSUM") as ph2_ps:
                for h in range(HPC):
                    p, off = h // 2, 64 * (h % 2)
                    at_ps = ph2_ps.tile([DK + 1, S], f32, tag="atp", bufs=1)
                    for j in range(NSB):
                        qstart = 128 * j if causal else 0
                        s0 = (qstart // 512) * 512
                        base = s0
                        while base < S:
                            w = min(1024, S - base)
                            a0 = (qstart - base) if base == s0 else 0
                            sc = ph2_ps.tile([128, 1024], f32, tag="sc", bufs=2)
                            a = a0
                            while a < w:
                                bnd = min((a // 512 + 1) * 512, w)
                                nc.tensor.matmul(
                                    sc[:, a:bnd],
                                    kt_pair[p][off:off + 64, 128 * j:128 * (j + 1)],
                                    qt_pair[p][off:off + 64, base + a:base + bnd],
                                    start=True, stop=True)
                                a = bnd
                            if causal and base == s0:
                                dc = qstart - base
                                nc.vector.tensor_tensor(
                                    out=sc[:, dc:dc + 128], in0=sc[:, dc:dc + 128],
                                    in1=mdiag_t, op=ADD)
                            if not causal:
                                mt = ph2_sb.tile([128, 1024], f32, tag="mt", bufs=3)
                                nc.sync.dma_start(
                                    out=mt[:, a0:w],
                                    in_=maskT_d[128 * j:128 * (j + 1), base + a0:base + w])
                                nc.vector.tensor_tensor(
                                    out=sc[:, a0:w], in0=sc[:, a0:w],
                                    in1=mt[:, a0:w], op=ADD)
                            pT = ph2_sb.tile([128, 1024], f32r, tag="pT", bufs=3)
                            nc.scalar.activation(out=pT[:, a0:w], in_=sc[:, a0:w], func=EXP)
                            a = a0
                            while a < w:
                                bnd = min((a // 512 + 1) * 512, w)
                                sbank = (base + a) // 512
                                last_j = min(NSB - 1, 4 * sbank + 3) if causal else NSB - 1
                                nc.tensor.matmul(at_ps[:, base + a:base + bnd],
                                                 v_sb[j][:, h, :], pT[:, a:bnd],
                                                 start=(j == 0), stop=(j == last_j))
                                a = bnd
                            base += w
                    # normalize: attnT_sb[p][off:off+64] = at_ps[0:64] / denom
                    rec = ph2_sb.tile([1, S], f32, tag="rec", bufs=1)
                    nc.vector.reciprocal(rec, at_ps[DK:DK + 1, :])
                    bc = ph2_sb.tile([64, S], f32, tag="bc", bufs=1)
                    nc.gpsimd.partition_broadcast(bc, rec)
                    nc.vector.tensor_tensor(out=attnT_sb[p][off:off + 64, :],
                                            in0=at_ps[0:DK, :], in1=bc, op=MULT)

            # =========================================================
            # Phase 3: output projection (partial; host sums cores + bias)
            # =========================================================
            with tc.tile_pool(name="ph3ps", bufs=1, space="PSUM") as ph3_ps:
                wo_t = [ph3_sb.tile([128, D], f32r, tag=f"wo{ch}", name=f"wo{ch}") for ch in range(NPAIR)]
                for ch in range(NPAIR):
                    nc.sync.dma_start(out=wo_t[ch], in_=wo_d[128 * ch:128 * (ch + 1), :])
                for si in range(NSB):
                    sl, sh = 128 * si, 128 * (si + 1)
                    for nh in range(2):
                        op = ph3_ps.tile([128, 512], f32, tag="op", bufs=3)
                        for ch in range(NPAIR):
                            nc.tensor.matmul(op, attnT_sb[ch][:, sl:sh],
                                             wo_t[ch][:, 512 * nh:512 * (nh + 1)],
                                             start=(ch == 0), stop=(ch == NPAIR - 1))
                        ob = ph3_sb.tile([128, 512], f32, tag="ob", bufs=3)
                        nc.scalar.copy(out=ob, in_=op)
                        nc.sync.dma_start(out=out_d[sl:sh, 512 * nh:512 * (nh + 1)], in_=ob)

    nc.compile()
    return nc


def _rope_tables():
    half = DK // 2
    freqs = (10000.0 ** (-2.0 / DK * np.arange(half, dtype=np.float32))).astype(np.float64)
    ang = np.outer(np.arange(S, dtype=np.float64), freqs)           # [S, 32]
    cos1 = np.cos(ang).T.astype(np.float32)                          # [32, S]
    sin1 = np.sin(ang).T.astype(np.float32)
    c64 = np.concatenate([cos1, cos1], axis=0)                       # [64, S]
    ssig64 = np.concatenate([sin1, -sin1], axis=0)                   # s-tilde(sigma(p))
    c128 = np.concatenate([c64, c64], axis=0)
    ssig128 = np.concatenate([ssig64, ssig64], axis=0)
    scale = np.float32(1.0 / math.sqrt(DK))
    return np.stack([c128 * scale, ssig128 * scale, c128, ssig128]).astype(np.float32)


def _psig():
    p64 = np.zeros((64, 64), np.float32)
    p64[np.arange(32) + 32, np.arange(32)] = 1.0
    p64[np.arange(32), np.arange(32) + 32] = 1.0
    p = np.zeros((128, 128), np.float32)
    p[0:64, 0:64] = p64
    p[64:128, 64:128] = p64
    return p


def kernel(x, mask, Wq, bq, Wk, bk, Wv, bv, Wo, bo):
    from concourse.bass_utils import run_bass_kernel_spmd

    x = np.asarray(x, dtype=np.float32)
    mask = np.asarray(mask)
    Wq, bq = np.asarray(Wq, np.float32), np.asarray(bq, np.float32)
    Wk, bk = np.asarray(Wk, np.float32), np.asarray(bk, np.float32)
    Wv, bv = np.asarray(Wv, np.float32), np.asarray(bv, np.float32)
    Wo, bo = np.asarray(Wo, np.float32), np.asarray(bo, np.float32)

    causal_ref = np.triu(np.ones((S, S), dtype=bool), k=1)
    m2 = np.broadcast_to(mask, (B, 1, S, S))[:, 0]
    causal = all(np.array_equal(m2[b], causal_ref) for b in range(B))

    key = causal
    if key not in _BUILD_CACHE:
        _BUILD_CACHE[key] = _build(causal)
    nc = _BUILD_CACHE[key]

    rope = _rope_tables()
    psig = _psig()
    ones = np.ones((1, 512), np.float32)
    mdiag = np.where(np.arange(128)[:, None] > np.arange(128)[None, :],
                     np.float32(-1e30), np.float32(0.0)).astype(np.float32)

    in_maps = []
    for c in range(NCORES):
        b, hg = c // (NCORES // B), c % (NCORES // B)
        cs = slice(DHC * hg, DHC * (hg + 1))
        im = {
            "xT": np.ascontiguousarray(x[b].T),
            "wq": np.ascontiguousarray(Wq[:, cs]),
            "wk": np.ascontiguousarray(Wk[:, cs]),
            "wv": np.ascontiguousarray(Wv[:, cs]),
            "wo": np.ascontiguousarray(Wo[cs, :]),
            "bqk": np.ascontiguousarray(np.stack([bq[cs], bk[cs]])),
            "bv": np.ascontiguousarray(bv[cs][None, :]),
            "ones": ones, "psig": psig, "rope": rope, "mdiag": mdiag,
        }
        if not causal:
            madd = np.where(m2[b], np.float32(-1e30), np.float32(0.0))
            im["maskT"] = np.ascontiguousarray(madd.T)
        in_maps.append(im)

    res = run_bass_kernel_spmd(nc, in_maps, core_ids=list(range(NCORES)))
    out = np.zeros((B, S, D), np.float32)
    for c in range(NCORES):
        out[c // (NCORES // B)] += res.results[c]["out"]
    out += bo[None, None, :]
    return out
